# revision 22
# baseline (speedup 1.0000x reference)
"""Two-layer GCN encoder on 8 Trainium2 NeuronCores (Bass/Tile), v3.

  out = Anorm @ relu(Anorm @ (x@W1) + b1) @ W2 + b2,  Anorm = D^-1/2 (A+I) D^-1/2

Everything runs in [chan (partitions), node (free)] orientation:

  phase A   g1T[:, v] = W1^T @ (dinv_v * x_v)     (x pre-scaled on host)
  AllGather four table quarters (overlapped with phase A / the agg passes)
  layer agg per (quarter, 4-block group):
      ap_gather   gt[chan, W]   edge-ordered source rows out of the SBUF
                                quarter table (edges sorted by dst)
      scan        ct = cumsum(gt) along free dim (leading zero column)
      ap_gather   bg[chan, 528] per-dst segment boundaries of ct
      sub/add     h_acc[:, dst] += ct[end_d] - ct[start_d]
  epilogue  h = dinv * relu(dinv * acc + b1)      (dinv broadcast tile)
  phase C   g2T[:, v] = W2^T @ h_v ; AllGather quarters; layer-2 agg same
  out       dinv * acc2 + b2, PE-transposed back to [node, chan]

Self-loops are ordinary edges; dinv[dst] and biases are applied in the
epilogues (bias is per-partition in this orientation).  No per-edge DMA
descriptors, no one-hot matmuls: the segment sums ride the DVE scan unit and
the Pool-engine SBUF gathers.
"""

import os

import numpy as np

import concourse.bass as bass
import concourse.bacc as bacc
import concourse.mybir as mybir
import concourse.tile as tile

P = 128
NQ = 4            # table quarters == aggregation passes
GB = 4            # dst blocks per group
BPB = 136       # boundary slots per block (129 used; 4*136/16 = 34 idx cols, 4B-aligned)

N_NODES = 100000
N_EDGES = 1600000
C_IN = 128
C_HID = 128
C_OUT = 64
N_CORES = 8


class Cfg:
    def __init__(self, n, cin, chid, cout, n_cores):
        assert n % n_cores == 0
        self.N = n
        self.CIN = cin
        self.CHID = chid
        self.COUT = cout
        self.NC = n_cores
        self.NPC = n // n_cores
        self.NBLK = -(-self.NPC // P)
        self.NPAD = self.NBLK * P
        assert self.NPC % NQ == 0
        self.QROWS = self.NPC // NQ          # local rows per quarter (3125)
        self.TROWS = self.QROWS * n_cores    # table rows per quarter (25000)
        assert self.TROWS <= 32767
        self.NGRP = -(-self.NBLK // GB)
        self.meta = None
        self.WMAX = None
        self.ICSUM = None
        self.ICQMAX = None


def _wrap16(idx):
    """ap_gather idx layout: idx i -> [16k + i%16, i//16], replicated k=0..7."""
    n = idx.shape[0]
    assert n % 16 == 0
    ic = n // 16
    out = np.zeros((P, ic), np.int16)
    i = np.arange(n)
    for k in range(8):
        out[16 * k + (i % 16), i // 16] = idx
    return out


def prep_inputs(cfg, x, edge_index, W1, b1, W2, b2):
    """One SPMD program: group widths W are the per-bucket MAX over cores;
    per-core data (gather idx, boundary positions) fills its own sizes."""
    NPC, QROWS = cfg.NPC, cfg.QROWS
    src = np.asarray(edge_index[0], dtype=np.int64)
    dst = np.asarray(edge_index[1], dtype=np.int64)
    deg = (np.bincount(dst, minlength=cfg.N) + 1.0).astype(np.float32)
    dinv = 1.0 / np.sqrt(deg)

    loops = np.arange(cfg.N, dtype=np.int64)
    src_all = np.concatenate([src, loops])
    dst_all = np.concatenate([dst, loops])
    order = np.argsort(dst_all, kind="stable")
    src_s = src_all[order]
    dst_s = dst_all[order]
    core_lo = np.searchsorted(dst_s, np.arange(cfg.NC) * NPC)
    core_hi = np.searchsorted(dst_s, (np.arange(cfg.NC) + 1) * NPC)

    x = np.asarray(x, dtype=np.float32)
    xs = x * dinv[:, None]
    W1 = np.asarray(W1, np.float32)
    b1 = np.asarray(b1, np.float32)
    W2 = np.asarray(W2, np.float32)
    b2 = np.asarray(b2, np.float32)

    nkey = NQ * cfg.NGRP * GB
    per_core = []
    counts = np.zeros((cfg.NC, nkey), np.int64)
    for c in range(cfg.NC):
        lo, hi = core_lo[c], core_hi[c]
        s1 = src_s[lo:hi]
        d1 = dst_s[lo:hi] - c * NPC
        blk = (d1 >> 7).astype(np.int64)
        sl = s1 % NPC
        q = sl // QROWS
        tidx = (s1 // NPC) * QROWS + (sl % QROWS)
        # sort edges by (q, grp, blk, dst) so each dst's edges are contiguous
        key = ((q * cfg.NGRP + blk // GB) * GB + (blk % GB)) * P + (d1 & 127)
        eorder = np.argsort(key, kind="stable")
        key_s = key[eorder] // P
        starts = np.searchsorted(key_s, np.arange(nkey + 1))
        counts[c] = starts[1:] - starts[:-1]
        # per-dst counts inside each block bucket
        per_core.append((tidx[eorder], (key[eorder] % P), starts))

    # common group widths (pad to %128 to keep scan tiles aligned; %16 for idx)
    gw = counts.max(axis=0).reshape(NQ * cfg.NGRP, GB).sum(axis=1)
    gw = ((gw + 127) // 128) * 128

    meta = []
    icoff = 0
    wmax = 128
    for qq in range(NQ):
        for g in range(cfg.NGRP):
            nb = min(GB, cfg.NBLK - g * GB)
            W = int(gw[qq * cfg.NGRP + g])
            meta.append(dict(q=qq, g=g, W=W, nb=nb, icoff=icoff))
            icoff += W // 16
            wmax = max(wmax, W)
    cfg.meta = meta
    cfg.WMAX = wmax
    cfg.ICSUM = max(icoff, 1)
    pass_ic = []
    for qq in range(NQ):
        lo = meta[qq * cfg.NGRP]["icoff"]
        hi = meta[(qq + 1) * cfg.NGRP]["icoff"] if qq < NQ - 1 else icoff
        pass_ic.append(hi - lo)
    cfg.ICQMAX = max(max(pass_ic), 1)
    cfg.BIC = GB * BPB // 16                 # boundary idx cols per group (33)
    cfg.BICQ = cfg.NGRP * cfg.BIC            # per pass

    maps = []
    for c in range(cfg.NC):
        tidx_s, dfine, starts = per_core[c]
        idx_all = np.zeros((P, cfg.ICSUM), np.int16)
        bidx_all = np.zeros((P, NQ * cfg.BICQ), np.int16)
        for mi, m in enumerate(meta):
            qq, g, W, nb = m["q"], m["g"], m["W"], m["nb"]
            ivv = np.zeros(W, np.int64)
            bpos = np.zeros(GB * BPB, np.int64)
            pos = 0
            for j in range(nb):
                k = (qq * cfg.NGRP + g) * GB + j
                a, bnd = starts[k], starts[k + 1]
                n = bnd - a
                ivv[pos:pos + n] = tidx_s[a:bnd]
                dcnt = np.bincount(dfine[a:bnd], minlength=P)
                cum = np.concatenate([[0], np.cumsum(dcnt)])  # [129]
                bpos[j * BPB: j * BPB + 129] = pos + cum
                pos += n
            idx_all[:, m["icoff"]:m["icoff"] + W // 16] = \
                _wrap16(ivv.astype(np.int16))
            bidx_all[:, mi * cfg.BIC:(mi + 1) * cfg.BIC] = \
                _wrap16(bpos.astype(np.int16))

        xsT = np.zeros((cfg.CIN, cfg.NPAD), np.float16)
        xsT[:, :NPC] = xs[c * NPC:(c + 1) * NPC].T
        dpad = np.ones(cfg.NPAD, np.float32)
        dpad[:NPC] = dinv[c * NPC:(c + 1) * NPC]
        dinvbc = np.tile(dpad[None, :], (P, 1)).astype(np.float16)

        maps.append({
            "xsT": xsT,
            "dinvbc": dinvbc,
            "idx_all": idx_all,
            "bidx_all": bidx_all,
            "w1": W1.astype(np.float16),
            "w2": W2.astype(np.float16),
            "b1col": b1.reshape(cfg.CHID, 1).astype(np.float32),
            "b2col": b2.reshape(cfg.COUT, 1).astype(np.float32),
        })
    return maps


def _agg_layer(nc, cfg, pools, tab_q, idx_dram, bidx_dram, acc, tchan, layer):
    """Segment-sum aggregation: gather -> cumsum -> boundary gather -> diff."""
    f32 = mybir.dt.float32
    OP = mybir.AluOpType
    tabp, idxp, gtp, ctp, bgp, dfp = pools
    meta = cfg.meta

    for q in range(NQ):
        tabq = tabp.tile([tchan, cfg.NC, cfg.QROWS], f32, tag=f"tab{layer}")
        nc.sync.dma_start(tabq[:], tab_q[q].transpose([1, 0, 2]))
        tabf = tabq[:].rearrange("p a b -> p (a b)")
        ic_lo = meta[q * cfg.NGRP]["icoff"]
        ic_hi = (meta[(q + 1) * cfg.NGRP]["icoff"] if q < NQ - 1 else cfg.ICSUM)
        ixt = idxp.tile([P, cfg.ICQMAX], mybir.dt.int16, tag="ixt")
        nc.sync.dma_start(ixt[:, :ic_hi - ic_lo], idx_dram.ap()[:, ic_lo:ic_hi])
        bxt = idxp.tile([P, cfg.BICQ], mybir.dt.int16, tag="bxt")
        nc.sync.dma_start(bxt[:],
                          bidx_dram.ap()[:, q * cfg.BICQ:(q + 1) * cfg.BICQ])

        for g in range(cfg.NGRP):
            m = meta[q * cfg.NGRP + g]
            W, nb = m["W"], m["nb"]
            io = m["icoff"] - ic_lo
            gt = gtp.tile([tchan, cfg.WMAX], f32, tag="gt")
            nc.gpsimd.ap_gather(
                gt[:, :W], tabf, ixt[:, io: io + W // 16],
                channels=tchan, num_elems=cfg.TROWS, d=1, num_idxs=W,
            )
            ct = ctp.tile([tchan, cfg.WMAX + 4], f32, tag="ct")
            nc.vector.memset(ct[:, 0:1], 0.0)
            nc.vector.tensor_tensor_scan(
                out=ct[:, 1:W + 1], data0=gt[:, :W], data1=gt[:, :W],
                initial=0.0, op0=OP.add, op1=OP.bypass,
            )
            bg = bgp.tile([tchan, GB * BPB], f32, tag="bg")
            nc.gpsimd.ap_gather(
                bg[:], ct[:, :W + 1], bxt[:, g * cfg.BIC:(g + 1) * cfg.BIC],
                channels=tchan, num_elems=W + 1, d=1, num_idxs=GB * BPB,
            )
            bg3 = bg.rearrange("p (a b) -> p a b", a=GB)
            df = dfp.tile([tchan, GB, P], f32, tag="df")
            nc.vector.tensor_tensor(
                out=df[:, :nb, :], in0=bg3[:, :nb, 1:129],
                in1=bg3[:, :nb, 0:128], op=OP.subtract,
            )
            nc.vector.tensor_tensor(
                out=acc[:, g * GB * P: g * GB * P + nb * P],
                in0=df[:].rearrange("p a b -> p (a b)")[:, :nb * P],
                in1=acc[:, g * GB * P: g * GB * P + nb * P],
                op=OP.add,
            )


def build_nc(cfg):
    nc = bacc.Bacc("TRN2", target_bir_lowering=False, debug=False,
                   num_devices=cfg.NC, num_swdge_queues=1)
    f32 = mybir.dt.float32
    f16 = mybir.dt.float16
    OP = mybir.AluOpType
    AF = mybir.ActivationFunctionType

    xsT = nc.dram_tensor("xsT", [cfg.CIN, cfg.NPAD], f16, kind="ExternalInput")
    dinvbc_t = nc.dram_tensor("dinvbc", [P, cfg.NPAD], f16,
                              kind="ExternalInput")
    idx_t = nc.dram_tensor("idx_all", [P, cfg.ICSUM], mybir.dt.int16,
                           kind="ExternalInput")
    bidx_t = nc.dram_tensor("bidx_all", [P, NQ * cfg.NGRP * GB * BPB // 16],
                            mybir.dt.int16, kind="ExternalInput")
    w1 = nc.dram_tensor("w1", [cfg.CIN, cfg.CHID], f16, kind="ExternalInput")
    w2 = nc.dram_tensor("w2", [cfg.CHID, cfg.COUT], f16, kind="ExternalInput")
    b1col = nc.dram_tensor("b1col", [cfg.CHID, 1], f32, kind="ExternalInput")
    b2col = nc.dram_tensor("b2col", [cfg.COUT, 1], f32, kind="ExternalInput")
    out = nc.dram_tensor("out", [cfg.NPC, cfg.COUT], f32, kind="ExternalOutput")
    h_dbg = nc.dram_tensor("h_dbg", [P, cfg.NPAD], f16, kind="ExternalOutput")
    a2_dbg = nc.dram_tensor("a2_dbg", [64, cfg.NPAD], f16, kind="ExternalOutput")

    with tile.TileContext(nc) as tc:
        with (
            tc.tile_pool(name="const", bufs=1) as constp,
            tc.tile_pool(name="xt", bufs=3) as xtp,
            tc.tile_pool(name="st", bufs=3) as stp,
            tc.tile_pool(name="dram", bufs=1, space="DRAM") as dramp,
        ):
            w1b = constp.tile([cfg.CIN, cfg.CHID], f16)
            nc.sync.dma_start(w1b[:], w1.ap())
            w2b = constp.tile([cfg.CHID, cfg.COUT], f16)
            nc.sync.dma_start(w2b[:], w2.ap())
            b1c = constp.tile([cfg.CHID, 1], f32)
            nc.sync.dma_start(b1c[:], b1col.ap())
            b2c = constp.tile([cfg.COUT, 1], f32)
            nc.sync.dma_start(b2c[:], b2col.ap())
            identh = constp.tile([P, P], f16)
            iota_i = constp.tile([P, 1], mybir.dt.int32)
            nc.gpsimd.iota(iota_i[:], pattern=[[1, 1]], base=0,
                           channel_multiplier=1)
            pidx_f = constp.tile([P, 1], f32)
            nc.vector.tensor_copy(pidx_f[:], iota_i[:])
            iota_r = constp.tile([P, P], mybir.dt.int32)
            nc.gpsimd.iota(iota_r[:], pattern=[[1, P]], base=0,
                           channel_multiplier=0)
            identf = constp.tile([P, P], f32)
            nc.vector.tensor_copy(identf[:], iota_r[:])
            nc.vector.tensor_scalar(out=identf[:], in0=identf[:],
                                    scalar1=pidx_f[:], scalar2=None,
                                    op0=OP.is_equal)

            g1_own = [dramp.tile([cfg.CHID, cfg.QROWS], f32,
                                 name=f"g1_own{q}") for q in range(NQ)]
            tab1 = [dramp.tile([cfg.NC, cfg.CHID, cfg.QROWS], f32,
                               addr_space="Shared", name=f"tab1_{q}")
                    for q in range(NQ)]
            g2_own = [dramp.tile([cfg.COUT, cfg.QROWS], f32,
                                 name=f"g2_own{q}") for q in range(NQ)]
            tab2 = [dramp.tile([cfg.NC, cfg.COUT, cfg.QROWS], f32,
                               addr_space="Shared", name=f"tab2_{q}")
                    for q in range(NQ)]

            # ---- phase A ---------------------------------------------------
            qfired = 0
            with tc.tile_pool(name="psA", bufs=4, space="PSUM") as psp:
                for b in range(cfg.NBLK):
                    rows = min(P, cfg.NPC - b * P)
                    xt = xtp.tile([cfg.CIN, P], f16, tag="xt")
                    nc.sync.dma_start(xt[:], xsT.ap()[:, b * P:(b + 1) * P])
                    ps = psp.tile([cfg.CHID, P], f32, tag="psA", space="PSUM")
                    nc.tensor.matmul(out=ps[:], lhsT=w1b[:], rhs=xt[:],
                                     start=True, stop=True)
                    st = stp.tile([cfg.CHID, P], f32, tag="stA")
                    nc.scalar.activation(st[:], ps[:], AF.Copy)
                    lo = b * P
                    hi = b * P + rows
                    while lo < hi:
                        q = lo // cfg.QROWS
                        qe = min(hi, (q + 1) * cfg.QROWS)
                        nc.sync.dma_start(
                            g1_own[q][:, lo - q * cfg.QROWS:
                                      qe - q * cfg.QROWS],
                            st[:, lo - b * P: qe - b * P])
                        lo = qe
                    while (qfired < NQ
                           and b * P + rows >= (qfired + 1) * cfg.QROWS):
                        nc.gpsimd.collective_compute(
                            "AllGather", OP.bypass,
                            replica_groups=[list(range(cfg.NC))],
                            ins=[g1_own[qfired].opt()],
                            outs=[tab1[qfired].opt()],
                        )
                        qfired += 1

            # ---- layer-1 aggregation --------------------------------------
            with tc.tile_pool(name="hacc", bufs=1) as haccp:
                h_acc = haccp.tile([P, cfg.NPAD], f16)
                nc.vector.memset(h_acc[:], 0.0)
                with (
                    tc.tile_pool(name="tab1p", bufs=1) as tabp,
                    tc.tile_pool(name="idx", bufs=1) as idxp,
                    tc.tile_pool(name="gt", bufs=2) as gtp,
                    tc.tile_pool(name="ct", bufs=2) as ctp,
                    tc.tile_pool(name="bgp", bufs=2) as bgp,
                    tc.tile_pool(name="dfp", bufs=2) as dfp,
                ):
                    pools = (tabp, idxp, gtp, ctp, bgp, dfp)
                    _agg_layer(nc, cfg, pools, tab1, idx_t, bidx_t, h_acc,
                               P, 1)

                # epilogue + phase C + AG2, quarter-chunked
                with (
                    tc.tile_pool(name="epi", bufs=3) as epip,
                    tc.tile_pool(name="dbc", bufs=1) as dbcp,
                    tc.tile_pool(name="psC", bufs=4, space="PSUM") as pspC,
                ):
                    dbc = dbcp.tile([P, cfg.NPAD], f16)
                    nc.sync.dma_start(dbc[:], dinvbc_t.ap())
                    ch_blocks = [(0, 25), (25, 50), (50, 75), (75, cfg.NBLK)]
                    for q, (blo, bhi) in enumerate(ch_blocks):
                        for b in range(blo, bhi):
                            rows = min(P, cfg.NPC - b * P)
                            sl = slice(b * P, b * P + P)
                            t1 = epip.tile([P, P], f32, tag="t1")
                            nc.vector.tensor_tensor(
                                out=t1[:], in0=h_acc[:, sl],
                                in1=dbc[:, sl], op=OP.mult)
                            t2 = epip.tile([P, P], f32, tag="t2")
                            nc.scalar.activation(t2[:], t1[:], AF.Relu,
                                                 bias=b1c[:])
                            nc.vector.tensor_tensor(
                                out=h_acc[:, sl], in0=t2[:],
                                in1=dbc[:, sl], op=OP.mult)
                            ps = pspC.tile([cfg.COUT, P], f32, tag="psC",
                                           space="PSUM")
                            nc.tensor.matmul(out=ps[:], lhsT=w2b[:],
                                             rhs=h_acc[:, sl], start=True,
                                             stop=True)
                            st = stp.tile([cfg.COUT, P], f32, tag="stC")
                            nc.scalar.activation(st[:], ps[:], AF.Copy)
                            lo = b * P
                            hi = b * P + rows
                            while lo < hi:
                                qq = lo // cfg.QROWS
                                qe = min(hi, (qq + 1) * cfg.QROWS)
                                nc.sync.dma_start(
                                    g2_own[qq][:, lo - qq * cfg.QROWS:
                                               qe - qq * cfg.QROWS],
                                    st[:, lo - b * P: qe - b * P])
                                lo = qe
                        nc.gpsimd.collective_compute(
                            "AllGather", OP.bypass,
                            replica_groups=[list(range(cfg.NC))],
                            ins=[g2_own[q].opt()], outs=[tab2[q].opt()],
                        )
                    nc.sync.dma_start(h_dbg.ap(), h_acc[:])

            # ---- layer-2 aggregation --------------------------------------
            with tc.tile_pool(name="acc2p", bufs=1) as acc2p:
                acc2 = acc2p.tile([64, cfg.NPAD], f16)
                nc.vector.memset(acc2[:], 0.0)
                with (
                    tc.tile_pool(name="tab2p", bufs=1) as tabp2,
                    tc.tile_pool(name="idx2", bufs=1) as idxp2,
                    tc.tile_pool(name="gt2", bufs=2) as gtp2,
                    tc.tile_pool(name="ct2", bufs=2) as ctp2,
                    tc.tile_pool(name="bgp2", bufs=2) as bgp2,
                    tc.tile_pool(name="dfp2", bufs=2) as dfp2,
                ):
                    pools = (tabp2, idxp2, gtp2, ctp2, bgp2, dfp2)
                    _agg_layer(nc, cfg, pools, tab2, idx_t, bidx_t, acc2,
                               64, 2)
                    nc.sync.dma_start(a2_dbg.ap(), acc2[:])

                with (
                    tc.tile_pool(name="epi2", bufs=3) as epi2p,
                    tc.tile_pool(name="dbc2", bufs=1) as dbc2p,
                    tc.tile_pool(name="psO", bufs=4, space="PSUM") as pspO,
                ):
                    dbc = dbc2p.tile([64, cfg.NPAD], f16)
                    nc.sync.dma_start(dbc[:], dinvbc_t.ap()[:64, :])
                    for b in range(cfg.NBLK):
                        rows = min(P, cfg.NPC - b * P)
                        sl = slice(b * P, b * P + P)
                        t1 = epi2p.tile([64, P], f32, tag="t1b")
                        nc.vector.tensor_tensor(
                            out=t1[:], in0=acc2[:, sl], in1=dbc[:, sl],
                            op=OP.mult)
                        t2 = epi2p.tile([64, P], f32, tag="t2b")
                        nc.vector.tensor_scalar(
                            out=t2[:], in0=t1[:], scalar1=b2c[:],
                            scalar2=None, op0=OP.add)
                        ps = pspO.tile([P, 64], f32, tag="psO", space="PSUM")
                        nc.tensor.matmul(out=ps[:], lhsT=t2[:],
                                         rhs=identf[:64, :64],
                                         start=True, stop=True,
                                         is_transpose=True)
                        ot = epi2p.tile([P, 64], f32, tag="ot")
                        nc.scalar.activation(ot[:], ps[:], AF.Copy)
                        nc.sync.dma_start(out.ap()[b * P: b * P + rows, :],
                                          ot[:rows, :])

    nc.compile()
    return nc


def run_cfg(cfg, inputs, ncs=None):
    from concourse import bass_utils

    maps = prep_inputs(
        cfg, inputs["x"], inputs["edge_index"], inputs["W1"], inputs["b1"],
        inputs["W2"], inputs["b2"],
    )
    nc = ncs if ncs else build_nc(cfg)

    kwargs = {}
    if os.environ.get("GCN_TRACE"):
        base = os.environ.get("GCN_TMPDIR")
        if base:
            os.makedirs(base, exist_ok=True)
        kwargs = dict(trace=True, tmpdir=base)

    res = bass_utils.run_bass_kernel_spmd(
        nc, maps, core_ids=list(range(cfg.NC)), **kwargs
    )
    outp = np.concatenate([res.results[c]["out"] for c in range(cfg.NC)],
                          axis=0)
    t = res.exec_time_ns
    return outp.astype(np.float32), (t, t, None)


def kernel(**inputs):
    cfg = Cfg(N_NODES, C_IN, C_HID, C_OUT, N_CORES)
    outp, _ = run_cfg(cfg, inputs)
    return outp
